# revision 1
# baseline (speedup 1.0000x reference)
"""EquivariantDenseAttention fused 8-core Trainium2 kernel.

Shards the N=4096 node/edge axis across 8 NeuronCores. Per core (R=512 edges):
  radial MLP -> tmp (f x b1) -> low-rank tensor-product bilinear (Khatri-Rao
  column tiles on PE, f32r) -> kqv -> AllGather(k, v) -> dense attention
  (transposed-score layout; row-max folded into the score matmul via an
  augmented contraction row; scores in bf16 hi/lo 2-pass for fp32-class
  accuracy; exp on ACT; PV + softmax denominator via an augmented ones
  column) -> equivariant projection epilogue.

Self-contained: hardcodes all shapes from the problem spec.
"""
import os
import numpy as np
import ml_dtypes

import concourse.bass as bass
import concourse.mybir as mybir
import concourse.tile as tile
from concourse import bacc
from concourse.bass_utils import run_bass_kernel_spmd

F32 = mybir.dt.float32
F32R = mybir.dt.float32r
BF16 = mybir.dt.bfloat16
AF = mybir.ActivationFunctionType
ALU = mybir.AluOpType

NC = 8                 # cores
N = 4096               # nodes/edges
R = N // NC            # 512 edges per core
ET = R // 128          # 4 e-tiles per core
NL, MULT, DIM = 2, 32, 4
HID = 128              # radial hidden dim = channel dim c
NH, HD = 8, 16         # heads, head dim
NJ = NL * MULT         # 64  (j index of tmp)
NI = 3 * MULT * NL     # 192 (i index of out1)
KT = N // 128          # 32 k-tiles over gathered edges
QT = R // 128          # 4 q-tiles per core
SG = 2                 # ktiles per exp group
VW = HD + 1            # v columns per head incl ones
REP_INDICES = np.array([0, 1, 1, 1])

_cache: dict = {}
SKIP = set(os.environ.get("KSKIP", "").split(",")) - {""}


def _build():
    nc = bacc.Bacc("TRN2", target_bir_lowering=False, debug=False, num_devices=NC)

    efa_x = nc.declare_dram_parameter("efa", [65, R], F32, isOutput=False)
    fe_x = nc.declare_dram_parameter("fe", [R, HID], F32, isOutput=False)
    b1s_x = nc.declare_dram_parameter("b1s", [R, 8], F32, isOutput=False)
    b2s_x = nc.declare_dram_parameter("b2s", [R, 8], F32, isOutput=False)
    w1a_x = nc.declare_dram_parameter("w1a", [65, HID], F32, isOutput=False)
    w2rt_x = nc.declare_dram_parameter("w2rt", [NJ, HID, NI], F32R, isOutput=False)
    b2t_x = nc.declare_dram_parameter("b2t", [NJ, NI], F32, isOutput=False)
    prsel_x = nc.declare_dram_parameter("prsel", [128, 128], F32, isOutput=False)
    pbias_x = nc.declare_dram_parameter("pbias", [128, 1], F32, isOutput=False)
    ident_x = nc.declare_dram_parameter("ident", [128, 128], F32, isOutput=False)
    identb_x = nc.declare_dram_parameter("identb", [128, 128], BF16, isOutput=False)
    out_x = nc.declare_dram_parameter("o_T", [128, R], F32, isOutput=True)

    # collective bounce buffers (k rows: [k_hi(16); k_lo(16)])
    kloc = nc.dram_tensor("kloc", [32, NH, R], BF16)
    kgat = nc.dram_tensor("kgat", [NC, 32, NH, R], BF16, addr_space="Shared")
    vloc = nc.dram_tensor("vloc", [R, NH * VW], BF16)
    vgat = nc.dram_tensor("vgat", [NC, R, NH * VW], BF16, addr_space="Shared")
    rgroups = [list(range(NC))]

    with tile.TileContext(nc, num_cores=NC) as tc:
        with tc.tile_pool(name="wpool", bufs=1) as wp:
            # ---- constants / weights ----
            w1a = wp.tile([65, HID], F32)
            nc.sync.dma_start(w1a[:], w1a_x.ap())
            ident = wp.tile([128, 128], F32)
            nc.sync.dma_start(ident[:], ident_x.ap())
            identb = wp.tile([128, 128], BF16)
            nc.sync.dma_start(identb[:], identb_x.ap())
            prsel = wp.tile([128, 128], F32)
            nc.sync.dma_start(prsel[:], prsel_x.ap())
            pbias = wp.tile([128, 1], F32)
            nc.sync.dma_start(pbias[:], pbias_x.ap())
            w2rt = wp.tile([128, NJ * NI], F32R)  # (128c, j*192+i)
            nc.sync.dma_start(
                w2rt[:].rearrange("c (j i) -> c j i", j=NJ),
                w2rt_x.ap().rearrange("j c i -> c j i"))
            b2tw = wp.tile([NJ, NI], F32)
            nc.sync.dma_start(b2tw[:], b2t_x.ap())

            efa = wp.tile([65, R], F32)
            nc.sync.dma_start(efa[:], efa_x.ap())
            fe_t = [wp.tile([128, HID], F32, tag=f"fe{t}", name=f"fe{t}") for t in range(ET)]
            b1sv = [wp.tile([128, 8], F32, tag=f"b1s{t}", name=f"b1s{t}") for t in range(ET)]
            b2sv = [wp.tile([128, 8], F32, tag=f"b2s{t}", name=f"b2s{t}") for t in range(ET)]
            for t in range(ET):
                sl = slice(t * 128, (t + 1) * 128)
                nc.sync.dma_start(fe_t[t][:], fe_x.ap()[sl])
                nc.sync.dma_start(b1sv[t][:], b1s_x.ap()[sl])
                nc.sync.dma_start(b2sv[t][:], b2s_x.ap()[sl])

            # ---- persistent attention operands ----
            qhi = wp.tile([16, NH * R], BF16)  # q_hi
            qB = wp.tile([33, NH * R], BF16)   # [q_lo(16); q_hi(16); -M(1)]
            kB = wp.tile([33, NH * N], BF16)   # [k_hi(16); k_lo(16); ones(1)]
            vts = [wp.tile([128, NH * VW], BF16, tag=f"vt{i}", name=f"vt{i}") for i in range(KT)]
            attnT = wp.tile([128, R], F32)

            # =========================================================
            # Phase 1: MLP + tmp + bilinear -> out1 -> kqv + local k,q,v
            # =========================================================
            with (
                tc.tile_pool(name="p1ps", bufs=1, space="PSUM") as pps,
                tc.tile_pool(name="p1sb", bufs=2) as psb,
                tc.tile_pool(name="gsb", bufs=4) as gsb,
                tc.tile_pool(name="o1ps", bufs=1, space="PSUM") as o1ps,
            ):
                # hT (128c, 512e) = relu(w1a.T @ efa)
                hT_ps = pps.tile([128, R], F32, tag="hT_ps")
                nc.tensor.matmul(hT_ps[:], w1a[:], efa[:], start=True, stop=True)
                hT = psb.tile([128, R], F32, tag="hT")
                nc.scalar.activation(hT[:], hT_ps[:], AF.Relu)

                # tmp_e (128e, 64j), j=(m,r)
                tmp_e = []
                for t in range(ET):
                    tm = psb.tile([128, NJ], F32, tag=f"tmp{t}")
                    for d in range(DIM):
                        for r in range(NL):
                            src = fe_t[t][:].rearrange("e (m d) -> e m d", d=DIM)[:, :, d]
                            dst = tm[:].rearrange("e (m r) -> e m r", r=NL)[:, :, r]
                            if d == 0:
                                nc.vector.tensor_scalar_mul(
                                    dst, src, b1sv[t][:, d * NL + r:d * NL + r + 1])
                            else:
                                nc.vector.scalar_tensor_tensor(
                                    dst, src, b1sv[t][:, d * NL + r:d * NL + r + 1],
                                    dst, op0=ALU.mult, op1=ALU.add)
                    tmp_e.append(tm)

                # tmpT (64j, 512e)
                tmpT = psb.tile([NJ, R], F32, tag="tmpT")
                for t in range(ET):
                    tp = pps.tile([NJ, 128], F32, tag="tmpT_ps")
                    nc.tensor.transpose(tp[:], tmp_e[t][:], ident[:])
                    nc.vector.tensor_copy(tmpT[:, t * 128:(t + 1) * 128], tp[:])

                # bilinear accumulation over j
                o1a = o1ps.tile([128, R], F32, tag="o1a")
                o1b = o1ps.tile([64, R], F32, tag="o1b")
                for j in range(2 if "bilin" in SKIP else NJ):
                    tstage = gsb.tile([1, R], F32, tag="tstage")
                    nc.sync.dma_start(tstage[:], tmpT[j:j + 1, :])
                    tbc = gsb.tile([128, R], F32, tag="tbc")
                    if "gbcast" in SKIP:
                        nc.vector.tensor_copy(tbc[:], hT[:])
                    else:
                        nc.gpsimd.partition_broadcast(
                            tbc[:], tstage[:], channels=128)
                    gsbt = gsb.tile([128, R], F32R, tag="G")
                    nc.vector.tensor_tensor(
                        gsbt[:], hT[:], tbc[:], op=ALU.mult)
                    wj = w2rt[:, j * NI:(j + 1) * NI]
                    nc.tensor.matmul(
                        o1a[:], wj[:, 0:128], gsbt[:],
                        start=(j == 0), stop=False)
                    nc.tensor.matmul(
                        o1b[:], wj[:, 128:NI], gsbt[:],
                        start=(j == 0), stop=False)
                nc.tensor.matmul(o1a[:], b2tw[:, 0:128], tmpT[:],
                                 start=False, stop=True)
                nc.tensor.matmul(o1b[:], b2tw[:, 128:NI], tmpT[:],
                                 start=False, stop=True)

                o1aT = psb.tile([128, R], F32, tag="o1aT")
                o1bT = psb.tile([64, R], F32, tag="o1bT")
                nc.scalar.copy(o1aT[:], o1a[:])
                nc.scalar.copy(o1bT[:], o1b[:])

                # staging tiles for q/k feature-part layouts (full R wide)
                khiT = psb.tile([128, R], BF16, tag="khiT")
                kloT = psb.tile([128, R], BF16, tag="kloT")
                qhiT = psb.tile([128, R], BF16, tag="qhiT")
                qloT = psb.tile([128, R], BF16, tag="qloT")

                for t in range(ET):
                    sl = slice(t * 128, (t + 1) * 128)
                    t1 = pps.tile([128, 128], F32, tag="o1tp_a")
                    nc.tensor.transpose(t1[:], o1aT[:, sl], ident[:])
                    t2 = pps.tile([128, 64], F32, tag="o1tp_b")
                    nc.tensor.transpose(t2[:], o1bT[:, sl], ident[0:64, 0:64])
                    o1e = psb.tile([128, NI], F32, tag="o1e")
                    nc.vector.tensor_copy(o1e[:, 0:128], t1[:])
                    nc.vector.tensor_copy(o1e[:, 128:NI], t2[:])

                    kqv = psb.tile([128, 3 * MULT * DIM], F32, tag="kqv")
                    for r in range(NL):
                        for d in range(DIM):
                            src = o1e[:].rearrange("e (x r) -> e x r", r=NL)[:, :, r]
                            dst = kqv[:].rearrange("e (x d) -> e x d", d=DIM)[:, :, d]
                            if r == 0:
                                nc.vector.tensor_scalar_mul(
                                    dst, src, b2sv[t][:, r * DIM + d:r * DIM + d + 1])
                            else:
                                nc.vector.scalar_tensor_tensor(
                                    dst, src, b2sv[t][:, r * DIM + d:r * DIM + d + 1],
                                    dst, op0=ALU.mult, op1=ALU.add)

                    k_e = kqv[:, 0:128]
                    q_e = kqv[:, 128:256]
                    v_e = kqv[:, 256:384]

                    # ---- v local (bf16, ones col per head) ----
                    vl = psb.tile([128, NH * VW], BF16, tag="vloc_t")
                    for hh in range(NH):
                        nc.vector.tensor_copy(
                            vl[:, hh * VW:hh * VW + HD],
                            v_e[:, hh * HD:(hh + 1) * HD])
                    nc.vector.memset(
                        vl[:].rearrange("e (h u) -> e h u", u=VW)[:, :, HD], 1.0)
                    nc.sync.dma_start(vloc.ap()[sl], vl[:])

                    # ---- k, q hi/lo splits (e-part) ----
                    k_hi = psb.tile([128, 128], BF16, tag="k_hi")
                    k_lo = psb.tile([128, 128], BF16, tag="k_lo")
                    nc.vector.tensor_copy(k_hi[:], k_e)
                    nc.vector.tensor_tensor(k_lo[:], k_e, k_hi[:], op=ALU.subtract)
                    q_s = psb.tile([128, 128], F32, tag="q_s")
                    nc.vector.tensor_scalar_mul(q_s[:], q_e, 0.25)
                    q_hi = psb.tile([128, 128], BF16, tag="q_hi")
                    q_lo = psb.tile([128, 128], BF16, tag="q_lo")
                    nc.vector.tensor_copy(q_hi[:], q_s[:])
                    nc.vector.tensor_tensor(q_lo[:], q_s[:], q_hi[:], op=ALU.subtract)

                    # transposes to feature-part (bf16), accumulate full-R tiles
                    for srct, dstt in ((k_hi, khiT), (k_lo, kloT),
                                       (q_hi, qhiT), (q_lo, qloT)):
                        tp = pps.tile([128, 128], BF16, tag="kq_tp", bufs=2)
                        nc.tensor.transpose(tp[:], srct[:], identb[:])
                        nc.vector.tensor_copy(dstt[:, sl], tp[:])

                # per-head DMAs: khiT/kloT -> kloc DRAM ; qhiT/qloT -> qA/qB
                for hh in range(NH):
                    hs = slice(hh * HD, (hh + 1) * HD)
                    nc.sync.dma_start(kloc.ap()[0:16, hh], khiT[hs, :])
                    nc.sync.dma_start(kloc.ap()[16:32, hh], kloT[hs, :])
                    qsl = slice(hh * R, (hh + 1) * R)
                    nc.sync.dma_start(qhi[0:16, qsl], qhiT[hs, :])
                    nc.sync.dma_start(qB[0:16, qsl], qloT[hs, :])
                    nc.sync.dma_start(qB[16:32, qsl], qhiT[hs, :])

            # ===================== collectives =====================
            nc.gpsimd.collective_compute(
                "AllGather", ALU.bypass, replica_groups=rgroups,
                ins=[kloc.ap().opt()], outs=[kgat.ap().opt()])
            nc.gpsimd.collective_compute(
                "AllGather", ALU.bypass, replica_groups=rgroups,
                ins=[vloc.ap().opt()], outs=[vgat.ap().opt()])

            # gather-back: kB rows = [k_hi; k_lo], kA rows = [k_hi; ones]
            nc.sync.dma_start(
                kB[0:32].rearrange("d (h c e) -> d h c e", h=NH, c=NC),
                kgat.ap().rearrange("c d h e -> d h c e"))
            nc.vector.memset(kB[32:33, :], 1.0)
            for i in range(KT):
                c, off = divmod(i, ET)
                nc.sync.dma_start(
                    vts[i][:], vgat.ap()[c][off * 128:(off + 1) * 128])

            kBh = kB[:].rearrange("d (h e) -> d h e", h=NH)
            qhih = qhi[:].rearrange("d (h e) -> d h e", h=NH)
            qBh = qB[:].rearrange("d (h e) -> d h e", h=NH)

            # ================= M-pass: row maxes =================
            with (
                tc.tile_pool(name="mps", bufs=1, space="PSUM") as mps,
                tc.tile_pool(name="mps2", bufs=1, space="PSUM") as mps2,
                tc.tile_pool(name="msb", bufs=2) as msb,
            ):
                for qt in range(QT):
                    qsl = slice(qt * 128, (qt + 1) * 128)
                    mcols = msb.tile([128, NH], F32, tag="mcols")
                    for hh in range(NH):
                        m2 = msb.tile([128, 2], F32, tag="m2")
                        for g in range(2):
                            sq = mps.tile([128, 2048], F32, tag="sq")
                            for kk in range(4):
                                ck = g * 4 + kk
                                nc.tensor.matmul(
                                    sq[:, kk * 512:(kk + 1) * 512],
                                    qhih[0:16, hh, qsl],
                                    kBh[0:16, hh, ck * 512:(ck + 1) * 512],
                                    start=True, stop=True)
                            nc.vector.tensor_reduce(
                                m2[:, g:g + 1], sq[:],
                                axis=mybir.AxisListType.X, op=ALU.max)
                        nc.vector.tensor_reduce(
                            mcols[:, hh:hh + 1], m2[:],
                            axis=mybir.AxisListType.X, op=ALU.max)
                    mt = mps2.tile([NH, 128], F32, tag="mt")
                    nc.tensor.transpose(mt[:], mcols[:], ident[:])
                    mneg = msb.tile([NH, 128], BF16, tag="mneg")
                    nc.vector.tensor_scalar_mul(mneg[:], mt[:], -1.0)
                    for hh in range(NH):
                        nc.sync.dma_start(
                            qBh[32:33, hh, qsl], mneg[hh:hh + 1, :])

            # ============= scores + exp + PV + normalize =============
            with (
                tc.tile_pool(name="sps", bufs=2, space="PSUM") as sps,
                tc.tile_pool(name="pvps", bufs=2, space="PSUM") as pvps,
                tc.tile_pool(name="ptsb", bufs=4) as ptsb,
                tc.tile_pool(name="nsb", bufs=2) as nsb,
            ):
                for hh in range(NH):
                    pv = pvps.tile([VW, R], F32, tag="pv")
                    for grp in range(KT // SG):
                        sg_t = sps.tile([128, SG * R], F32, tag="sg")
                        for u in range(SG):
                            kt = grp * SG + u
                            ksl = slice(kt * 128, (kt + 1) * 128)
                            reg = sg_t[:, u * R:(u + 1) * R]
                            nc.tensor.matmul(
                                reg, kBh[0:16, hh, ksl], qhih[0:16, hh, :],
                                start=True, stop=False)
                            nc.tensor.matmul(
                                reg, kBh[:, hh, ksl], qBh[:, hh, :],
                                start=False, stop=True)
                        pt = ptsb.tile([128, SG * R], BF16, tag="pt")
                        nc.scalar.activation(pt[:], sg_t[:], AF.Exp)
                        for u in range(SG):
                            kt = grp * SG + u
                            nc.tensor.matmul(
                                pv[:], vts[kt][:, hh * VW:(hh + 1) * VW],
                                pt[:, u * R:(u + 1) * R],
                                start=(kt == 0), stop=(kt == KT - 1))
                    pvsb = nsb.tile([VW, R], F32, tag="pvsb")
                    nc.vector.tensor_copy(pvsb[:], pv[:])
                    den = nsb.tile([1, R], F32, tag="den")
                    nc.sync.dma_start(den[:], pvsb[HD:HD + 1, :])
                    rec = nsb.tile([1, R], F32, tag="rec")
                    nc.vector.reciprocal(rec[:], den[:])
                    rec16 = nsb.tile([HD, R], F32, tag="rec16")
                    nc.gpsimd.partition_broadcast(rec16[:], rec[:], channels=HD)
                    prod = nsb.tile([HD, R], F32, tag="prod")
                    nc.vector.tensor_tensor(
                        prod[:], pvsb[0:HD, :], rec16[:], op=ALU.mult)
                    nc.sync.dma_start(attnT[hh * HD:(hh + 1) * HD, :], prod[:])

            # ================= projection epilogue =================
            with (
                tc.tile_pool(name="ops", bufs=1, space="PSUM") as ops,
                tc.tile_pool(name="osb", bufs=1) as osb,
            ):
                op_ps = ops.tile([128, R], F32, tag="oproj")
                nc.tensor.matmul(op_ps[:], prsel[:], attnT[:], start=True, stop=True)
                ot = osb.tile([128, R], F32, tag="ot")
                nc.vector.tensor_scalar_add(ot[:], op_ps[:], pbias[:, 0:1])
                nc.sync.dma_start(out_x.ap(), ot[:])

    nc.compile()
    return nc


def _stage(inputs):
    b1 = np.asarray(inputs["b1"], np.float32)
    b2 = np.asarray(inputs["b2"], np.float32)
    ef = np.asarray(inputs["edge_feats"], np.float32)
    f = np.asarray(inputs["f"], np.float32)
    w1 = np.asarray(inputs["w1"], np.float32)
    bias1 = np.asarray(inputs["bias1"], np.float32)
    w2 = np.asarray(inputs["w2"], np.float32)
    bias2 = np.asarray(inputs["bias2"], np.float32)
    proj_w = np.asarray(inputs["proj_w"], np.float32)
    proj_b = np.asarray(inputs["proj_b"], np.float32)
    src_idx = np.asarray(inputs["src_idx"])

    f_src = f[src_idx]                                   # edge gather
    efa = np.concatenate([ef.T, np.ones((1, N), np.float32)])       # (65, N)
    w1a = np.concatenate([w1.T, bias1[None, :]]).astype(np.float32)  # (65, 128)
    w2rt = np.ascontiguousarray(
        w2.reshape(NI, NJ, HID).transpose(1, 2, 0)).astype(np.float32)
    b2t = np.ascontiguousarray(bias2.reshape(NI, NJ).T).astype(np.float32)
    prsel = np.zeros((MULT * DIM, MULT * DIM), np.float32)
    for m in range(MULT):
        for mp in range(MULT):
            for d in range(DIM):
                prsel[m * DIM + d, mp * DIM + d] = proj_w[REP_INDICES[d] * MULT + mp, m]
    pbias = np.zeros((MULT * DIM, 1), np.float32)
    pbias[0::DIM, 0] = proj_b[:, 0]
    ident = np.eye(128, dtype=np.float32)
    identb = np.eye(128, dtype=ml_dtypes.bfloat16)

    in_maps = []
    for c in range(NC):
        sl = slice(c * R, (c + 1) * R)
        in_maps.append({
            "efa": np.ascontiguousarray(efa[:, sl]),
            "fe": np.ascontiguousarray(f_src[sl].reshape(R, HID)),
            "b1s": np.ascontiguousarray(b1[sl].reshape(R, 8)),
            "b2s": np.ascontiguousarray(b2[sl].reshape(R, 8)),
            "w1a": w1a, "w2rt": w2rt, "b2t": b2t,
            "prsel": prsel, "pbias": pbias,
            "ident": ident, "identb": identb,
        })
    return in_maps


def last_exec_time_ns():
    return _cache.get("exec_time_ns")


def kernel(**inputs):
    if "nc" not in _cache:
        _cache["nc"] = _build()
    nc = _cache["nc"]
    in_maps = _stage(inputs)
    res = run_bass_kernel_spmd(nc, in_maps, core_ids=list(range(NC)))
    _cache["exec_time_ns"] = res.exec_time_ns
    out = np.empty((N, MULT, DIM), np.float32)
    for c in range(NC):
        out[c * R:(c + 1) * R] = res.results[c]["o_T"].T.reshape(R, MULT, DIM)
    return out

